# revision 19
# baseline (speedup 1.0000x reference)
"""Trainium2 Bass kernel for AnisotropicGaussianSampler.

Reference computation (H=W=128, N=4096 samples, B=8):
    corr[b,n] = (1/(H*W)) * sum_{h,w} A[b,h,w] * exp(-eh[h,n]) * exp(-ew[w,n])
    eh[h,n] = (h/H - mu[n,0])^2 / (2*sigma[n,0]^2)   (separable in h and w)

Factorization used on-device (per sample column n):
    Ph[h,n] = exp(-0.5 * zh^2),  zh = (mu0[n] - h/H) / sigma0[n]
    Pw[w,n] = exp(-0.5 * zw^2)
    N_b[w,n] = sum_h A[b,h,w] * Ph[h,n]          (matmul, lhsT = A_b as stored)
    corr[b,n] = (1/(H*W)) * sum_w Pw[w,n]*N_b[w,n]  (mul + ones-reduce matmul)

Table prep: 1/sigma and mu/sigma are computed across 128 partitions (fast DVE)
in a [128, (t,q,c)] column tile, PE-transposed to [16, 128] in one shot, copied
to SBUF, and DMA-gathered into one [2, 512] row tile per axis. A single K=2
matmul per axis (constant lhsT rows {ones, -grid}) then produces z in PSUM:
z[h,n] = (mu/sigma)[n] - (h/H)*(1/sigma)[n]; DVE squares it and ACT exps it.

DMA routing: the latency-critical small loads go on the sync HWDGE ring; the
512KB activations load is a single DMA on the scalar HWDGE ring so both rings
run in parallel (each InstDMACopy is split across all 16 SDMA engines).

The batch loop is software-pipelined (skew 2) so the DVE multiply of batch b
overlaps the mm1 matmuls of batches b+1/b+2; the final reduce accumulates all
8 batches into one [8,512] PSUM tile via per-batch one-hot lhsT columns.

Sharding: the 4096 sample points are split 512-per-core across 8 NeuronCores
(data-parallel in n); every core gets the full activations. Host concatenates
the per-core [8,512] outputs. No collectives needed.
"""

import os
import sys

import numpy as np

if "/opt/trn_rl_repo" not in sys.path:
    sys.path.insert(0, "/opt/trn_rl_repo")

B, H, W = 8, 128, 128
N_TOTAL = 4096
N_CORES = 8
NS = N_TOTAL // N_CORES  # 512 samples per core
NCH = NS // 128          # n-chunks per core (4)

# matmul input dtype mode: "f32r" (single-pass fp32) or "f32" (4x slower)
MM_MODE = os.environ.get("KERNEL_MM_MODE", "f32r")

LAST_EXEC_TIME_NS = None

_CACHE = {}


def _build_bass():
    import concourse.mybir as mybir
    import concourse.tile as tile
    from concourse import bacc

    f32 = mybir.dt.float32
    mmdt = mybir.dt.float32r if MM_MODE == "f32r" else f32

    nc = bacc.Bacc()

    acts_d = nc.declare_dram_parameter("activations", [B, H, W], mmdt, isOutput=False)
    mu_d = nc.declare_dram_parameter("mu", [NS, 2], f32, isOutput=False)
    sig_d = nc.declare_dram_parameter("sigma", [NS, 2], f32, isOutput=False)
    # zconst rows: {ones(H), -grid(H)}
    zconst_d = nc.declare_dram_parameter("zconst", [2, H], mmdt, isOutput=False)
    oneh_d = nc.declare_dram_parameter("onehots", [W, B * B], mmdt, isOutput=False)
    ident_d = nc.declare_dram_parameter("ident", [128, 128], f32, isOutput=False)
    out_d = nc.declare_dram_parameter("out", [B, NS], f32, isOutput=True)

    Exp = mybir.ActivationFunctionType.Exp
    Square = mybir.ActivationFunctionType.Square

    with tile.TileContext(nc) as tc, nc.allow_low_precision(
        reason="float32r matmul inputs carry ~f32 precision"
    ):
        with (
            tc.tile_pool(name="const", bufs=1) as constp,
            tc.tile_pool(name="io", bufs=1) as iop,
            tc.tile_pool(name="sq", bufs=2) as sqp,
            tc.tile_pool(name="vbuf", bufs=4) as vp,
            tc.tile_pool(name="psz", bufs=2, space="PSUM") as pszp,
            tc.tile_pool(name="pst", bufs=1, space="PSUM") as pstp,
            tc.tile_pool(name="psn", bufs=4, space="PSUM") as psnp,
            tc.tile_pool(name="pso", bufs=1, space="PSUM") as psop,
        ):
            # ---- critical-path loads on the sync HWDGE ring ----
            mu_cols = iop.tile([128, 2, NCH], f32)
            nc.sync.dma_start(
                mu_cols[:], mu_d[:].rearrange("(c p) t -> p t c", p=128)
            )
            sig_cols = iop.tile([128, 2, NCH], f32)
            nc.sync.dma_start(
                sig_cols[:], sig_d[:].rearrange("(c p) t -> p t c", p=128)
            )
            ident = constp.tile([128, 128], f32)
            nc.sync.dma_start(ident[:], ident_d[:])
            zconst = constp.tile([2, H], mmdt)
            nc.sync.dma_start(zconst[:], zconst_d[:])
            oneh = constp.tile([W, B * B], mmdt)
            nc.sync.dma_start(oneh[:], oneh_d[:])

            # ---- activations: one 512KB DMA on the scalar HWDGE ring ----
            acts_sb = iop.tile([H, B, W], mmdt)
            nc.scalar.dma_start(acts_sb[:], acts_d[:].rearrange("b h w -> h b w"))

            # ---- prep columns [128, (q, t, c)], q in {mu/sigma, 1/sigma} ----
            cols = iop.tile([128, 2, 2, NCH], f32)
            nc.vector.reciprocal(cols[:, 1, :, :], sig_cols[:])
            nc.vector.tensor_mul(cols[:, 0, :, :], mu_cols[:], cols[:, 1, :, :])

            # transpose all 16 columns at once -> [16, 128] rows
            tps = pstp.tile([2 * 2 * NCH, 128], f32)
            nc.tensor.transpose(
                tps[:], cols[:].rearrange("p q t c -> p (q t c)"), ident[:]
            )
            tsb = iop.tile([2 * 2 * NCH, 128], mmdt)
            nc.scalar.copy(tsb[:], tps[:])

            # gather one [2, NS] row tile per axis: rows {mu/sigma, 1/sigma};
            # one DMA per (q, t) — row q of zr gets tsb rows (q, t, 0..3)
            zrows = []
            for t in range(2):
                zr = iop.tile([2, NS], mmdt, tag=f"zr{t}", name=f"zr{t}")
                for q in range(2):
                    j = (q * 2 + t) * NCH
                    eng = nc.sync if t == 0 else nc.scalar
                    eng.dma_start(
                        zr[q:q + 1, :].rearrange("one (c p) -> one c p", c=NCH),
                        tsb[j:j + NCH, :],
                    )
                zrows.append(zr)

            # ---- z = K=2 matmul; square on DVE; exp on ACT ----
            Ph = iop.tile([H, NS], mmdt)
            Pw = iop.tile([W, NS], mmdt)
            for t, ptab in ((0, Ph), (1, Pw)):
                ps_z = pszp.tile([H, NS], f32, tag="z", name=f"ps_z{t}")
                nc.tensor.matmul(
                    ps_z[:], lhsT=zconst[:], rhs=zrows[t][:], start=True, stop=True
                )
                sq = sqp.tile([H, NS], f32, tag="sq", name=f"sq{t}")
                nc.scalar.activation(sq[:], ps_z[:], Square)
                nc.scalar.activation(ptab[:], sq[:], Exp, scale=-0.5)

            # ---- pipelined batch loop (skew 2) ----
            SKEW = 2
            ps_out = psop.tile([B, NS], f32)
            ps_n = [None] * B

            def mm1(b):
                ps_n[b] = psnp.tile([W, NS], f32, tag="n", name=f"ps_n{b}")
                nc.tensor.matmul(
                    ps_n[b][:], lhsT=acts_sb[:, b, :], rhs=Ph[:],
                    start=True, stop=True,
                )

            for b in range(SKEW):
                mm1(b)
            for b in range(B):
                if b + SKEW < B:
                    mm1(b + SKEW)
                v = vp.tile([W, NS], mmdt, tag="v", name=f"v{b}")
                nc.vector.tensor_mul(v[:], ps_n[b][:], Pw[:])
                nc.tensor.matmul(
                    ps_out[:], lhsT=oneh[:, b * B:(b + 1) * B], rhs=v[:],
                    start=(b == 0), stop=(b == B - 1),
                )

            # ---- scale by 1/(H*W), store ----
            out_sb = iop.tile([B, NS], f32)
            nc.scalar.mul(out_sb[:], ps_out[:], 1.0 / (H * W))
            nc.sync.dma_start(out_d[:], out_sb[:])

    nc.compile()
    return nc


def _constants():
    gh = np.arange(H, dtype=np.float32) / H
    zconst = np.ascontiguousarray(
        np.stack([np.ones(H, np.float32), -gh]).astype(np.float32)
    )
    oneh = np.zeros((W, B * B), np.float32)
    for b in range(B):
        oneh[:, b * B + b] = 1.0
    ident = np.eye(128, dtype=np.float32)
    return zconst, oneh, ident


def kernel(activations, mu, sigma):
    from concourse.bass_utils import run_bass_kernel_spmd

    global LAST_EXEC_TIME_NS

    activations = np.ascontiguousarray(np.asarray(activations, dtype=np.float32))
    mu = np.ascontiguousarray(np.asarray(mu, dtype=np.float32))
    sigma = np.ascontiguousarray(np.asarray(sigma, dtype=np.float32))
    assert activations.shape == (B, H, W)
    assert mu.shape == (N_TOTAL, 2) and sigma.shape == (N_TOTAL, 2)

    if "nc" not in _CACHE:
        _CACHE["nc"] = _build_bass()
    nc = _CACHE["nc"]

    zconst, oneh, ident = _constants()
    in_maps = []
    for c in range(N_CORES):
        sl = slice(c * NS, (c + 1) * NS)
        in_maps.append(
            {
                "activations": activations,
                "mu": np.ascontiguousarray(mu[sl]),
                "sigma": np.ascontiguousarray(sigma[sl]),
                "zconst": zconst,
                "onehots": oneh,
                "ident": ident,
            }
        )

    res = run_bass_kernel_spmd(nc, in_maps, core_ids=list(range(N_CORES)))
    LAST_EXEC_TIME_NS = res.exec_time_ns

    out = np.concatenate([r["out"] for r in res.results], axis=1)  # [B, N_TOTAL]
    return out.reshape(B, 64, 64).astype(np.float32)


# revision 20
# speedup vs baseline: 1.2430x; 1.2430x over previous
"""Trainium2 Bass kernel for AnisotropicGaussianSampler.

Reference computation (H=W=128, N=4096 samples, B=8):
    corr[b,n] = (1/(H*W)) * sum_{h,w} A[b,h,w] * exp(-eh[h,n]) * exp(-ew[w,n])
    eh[h,n] = (h/H - mu[n,0])^2 / (2*sigma[n,0]^2)   (separable in h and w)

Factorization used on-device (per sample column n):
    Ph[h,n] = exp(-0.5 * zh^2),  zh = (mu0[n] - h/H) / sigma0[n]
    Pw[w,n] = exp(-0.5 * zw^2)
    N_b[w,n] = sum_h A[b,h,w] * Ph[h,n]          (matmul, lhsT = A_b as stored)
    corr[b,n] = (1/(H*W)) * sum_w Pw[w,n]*N_b[w,n]  (mul + ones-reduce matmul)

Precision split: the z tables are produced in float32r (single-pass fp32
matmul; z is cancellation-sensitive), while the big batch matmuls run in
float16 (same 1 cycle/row as f32r but ~10x faster weight loads via FWL;
fp16's 11-bit mantissa keeps the result within ~2e-3).

Table prep: 1/sigma and mu/sigma are computed across 128 partitions (fast DVE)
in a [128, (q,t,c)] column tile, PE-transposed to [16, 128] in one shot, copied
to SBUF (rounding to f32r), and DMA-gathered into one [2, 512] row tile per
axis. A single K=2 matmul per axis (constant lhsT rows {ones, -grid}) then
produces z in PSUM; ACT squares and exponentiates it.

DMA routing: all small loads are packed into ONE [128, 144] bundle (mu, sigma,
identity) on the sync HWDGE ring; zconst/onehots ride the scalar ring; the
512KB activations load is a single casting DMA (f32 -> f16) on gpsimd SWDGE.

The batch loop is software-pipelined (skew 2) so the DVE multiply of batch b
overlaps the mm1 matmuls of batches b+1/b+2; the final reduce accumulates all
8 batches into one [8,512] PSUM tile via per-batch one-hot lhsT columns.

Sharding: the 4096 sample points are split 512-per-core across 8 NeuronCores
(data-parallel in n); every core gets the full activations. Host concatenates
the per-core [8,512] outputs. No collectives needed.
"""

import os
import sys

import numpy as np

if "/opt/trn_rl_repo" not in sys.path:
    sys.path.insert(0, "/opt/trn_rl_repo")

B, H, W = 8, 128, 128
N_TOTAL = 4096
N_CORES = 8
NS = N_TOTAL // N_CORES  # 512 samples per core
NCH = NS // 128          # n-chunks per core (4)

LAST_EXEC_TIME_NS = None

_CACHE = {}


def _build_bass():
    import concourse.mybir as mybir
    import concourse.tile as tile
    from concourse import bacc

    f32 = mybir.dt.float32
    f32r = mybir.dt.float32r
    f16 = mybir.dt.float16

    nc = bacc.Bacc()

    acts_d = nc.declare_dram_parameter("activations", [B, H, W], f32, isOutput=False)
    # bundle columns: [mu (t,c): 8 | sigma (t,c): 8 | identity: 128]
    bund_d = nc.declare_dram_parameter("bundle", [128, 144], f32, isOutput=False)
    # zconst rows: {ones(H), -grid(H)}
    zconst_d = nc.declare_dram_parameter("zconst", [2, H], f32r, isOutput=False)
    oneh_d = nc.declare_dram_parameter("onehots", [W, B * B], f16, isOutput=False)
    out_d = nc.declare_dram_parameter("out", [B, NS], f32, isOutput=True)

    Exp = mybir.ActivationFunctionType.Exp
    Square = mybir.ActivationFunctionType.Square

    with tile.TileContext(nc) as tc, nc.allow_low_precision(
        reason="float32r/f16 matmul inputs are intentional"
    ):
        with (
            tc.tile_pool(name="const", bufs=1) as constp,
            tc.tile_pool(name="io", bufs=1) as iop,
            tc.tile_pool(name="sq", bufs=2) as sqp,
            tc.tile_pool(name="vbuf", bufs=4) as vp,
            tc.tile_pool(name="psz", bufs=2, space="PSUM") as pszp,
            tc.tile_pool(name="pst", bufs=1, space="PSUM") as pstp,
            tc.tile_pool(name="psn", bufs=4, space="PSUM") as psnp,
            tc.tile_pool(name="pso", bufs=1, space="PSUM") as psop,
        ):
            # ---- loads: bundle on sync ring, consts on scalar, acts on gpsimd ----
            bund = constp.tile([128, 144], f32)
            nc.sync.dma_start(bund[:], bund_d[:])
            mu_cols = bund[:, 0:8].rearrange("p (t c) -> p t c", c=NCH)
            sig_cols = bund[:, 8:16].rearrange("p (t c) -> p t c", c=NCH)
            ident = bund[:, 16:144]

            zconst = constp.tile([2, H], f32r)
            nc.scalar.dma_start(zconst[:], zconst_d[:])
            oneh = constp.tile([W, B * B], f16)
            nc.scalar.dma_start(oneh[:], oneh_d[:])

            acts_sb = iop.tile([H, B, W], f16)
            nc.gpsimd.dma_start(acts_sb[:], acts_d[:].rearrange("b h w -> h b w"))

            # ---- prep columns [128, (q, t, c)], q in {mu/sigma, 1/sigma} ----
            cols = iop.tile([128, 2, 2, NCH], f32)
            nc.vector.reciprocal(cols[:, 1, :, :], sig_cols)
            nc.vector.tensor_mul(cols[:, 0, :, :], mu_cols, cols[:, 1, :, :])

            # transpose all 16 columns at once -> [16, 128] rows
            tps = pstp.tile([2 * 2 * NCH, 128], f32)
            nc.tensor.transpose(
                tps[:], cols[:].rearrange("p q t c -> p (q t c)"), ident
            )
            tsb = iop.tile([2 * 2 * NCH, 128], f32r)
            nc.scalar.copy(tsb[:], tps[:])

            # gather one [2, NS] row tile per axis: rows {mu/sigma, 1/sigma};
            # one DMA per (q, t) — row q of zr gets tsb rows (q, t, 0..3)
            zrows = []
            for t in range(2):
                zr = iop.tile([2, NS], f32r, tag=f"zr{t}", name=f"zr{t}")
                for q in range(2):
                    j = (q * 2 + t) * NCH
                    eng = nc.sync if t == 0 else nc.scalar
                    eng.dma_start(
                        zr[q:q + 1, :].rearrange("one (c p) -> one c p", c=NCH),
                        tsb[j:j + NCH, :],
                    )
                zrows.append(zr)

            # ---- z = K=2 matmul (f32r); square+exp on ACT ----
            Ph = iop.tile([H, NS], f16)
            Pw = iop.tile([W, NS], f32)
            for t, ptab in ((0, Ph), (1, Pw)):
                ps_z = pszp.tile([H, NS], f32, tag="z", name=f"ps_z{t}")
                nc.tensor.matmul(
                    ps_z[:], lhsT=zconst[:], rhs=zrows[t][:], start=True, stop=True
                )
                sq = sqp.tile([H, NS], f32, tag="sq", name=f"sq{t}")
                nc.scalar.activation(sq[:], ps_z[:], Square)
                nc.scalar.activation(ptab[:], sq[:], Exp, scale=-0.5)

            # ---- pipelined batch loop (skew 2), fp16 matmuls ----
            SKEW = 2
            ps_out = psop.tile([B, NS], f32)
            ps_n = [None] * B

            def mm1(b):
                ps_n[b] = psnp.tile([W, NS], f32, tag="n", name=f"ps_n{b}")
                nc.tensor.matmul(
                    ps_n[b][:], lhsT=acts_sb[:, b, :], rhs=Ph[:],
                    start=True, stop=True,
                )

            for b in range(SKEW):
                mm1(b)
            for b in range(B):
                if b + SKEW < B:
                    mm1(b + SKEW)
                v = vp.tile([W, NS], f16, tag="v", name=f"v{b}")
                nc.vector.tensor_mul(v[:], ps_n[b][:], Pw[:])
                nc.tensor.matmul(
                    ps_out[:], lhsT=oneh[:, b * B:(b + 1) * B], rhs=v[:],
                    start=(b == 0), stop=(b == B - 1),
                )

            # ---- scale by 1/(H*W), store ----
            out_sb = iop.tile([B, NS], f32)
            nc.scalar.mul(out_sb[:], ps_out[:], 1.0 / (H * W))
            nc.sync.dma_start(out_d[:], out_sb[:])

    nc.compile()
    return nc


def _constants():
    gh = np.arange(H, dtype=np.float32) / H
    zconst = np.ascontiguousarray(
        np.stack([np.ones(H, np.float32), -gh]).astype(np.float32)
    )
    oneh = np.zeros((W, B * B), np.float16)
    for b in range(B):
        oneh[:, b * B + b] = 1.0
    ident = np.eye(128, dtype=np.float32)
    return zconst, oneh, ident


def _bundle(mu_sl, sig_sl, ident):
    # [128, 8 | 8 | 128]: mu/sigma in (t, c) column order, then identity
    mu_cols = mu_sl.reshape(NCH, 128, 2).transpose(1, 2, 0).reshape(128, 8)
    sig_cols = sig_sl.reshape(NCH, 128, 2).transpose(1, 2, 0).reshape(128, 8)
    return np.ascontiguousarray(
        np.concatenate([mu_cols, sig_cols, ident], axis=1).astype(np.float32)
    )


def kernel(activations, mu, sigma):
    from concourse.bass_utils import run_bass_kernel_spmd

    global LAST_EXEC_TIME_NS

    activations = np.ascontiguousarray(np.asarray(activations, dtype=np.float32))
    mu = np.ascontiguousarray(np.asarray(mu, dtype=np.float32))
    sigma = np.ascontiguousarray(np.asarray(sigma, dtype=np.float32))
    assert activations.shape == (B, H, W)
    assert mu.shape == (N_TOTAL, 2) and sigma.shape == (N_TOTAL, 2)

    if "nc" not in _CACHE:
        _CACHE["nc"] = _build_bass()
    nc = _CACHE["nc"]

    zconst, oneh, ident = _constants()
    in_maps = []
    for c in range(N_CORES):
        sl = slice(c * NS, (c + 1) * NS)
        in_maps.append(
            {
                "activations": activations,
                "bundle": _bundle(mu[sl], sigma[sl], ident),
                "zconst": zconst,
                "onehots": oneh,
            }
        )

    res = run_bass_kernel_spmd(nc, in_maps, core_ids=list(range(N_CORES)))
    LAST_EXEC_TIME_NS = res.exec_time_ns

    out = np.concatenate([r["out"] for r in res.results], axis=1)  # [B, N_TOTAL]
    return out.reshape(B, 64, 64).astype(np.float32)
